# revision 34
# baseline (speedup 1.0000x reference)
"""Trainium2 Bass kernel for: out = conv3x3(x, weight*A_w) * sigmoid(conv3x3(relu(conv3x3(x, se_w1)), se_w2))

Sharding: data-parallel over batch B=8 -> 8 NeuronCores (one image per core);
weight / A_w / se_w1 / se_w2 replicated to every core. The conv weights are
passed transposed to [ci, kh, kw, co] (host-side layout prep during sharding)
so the matmul stationary operand loads straight from DRAM.

Per-core kernel (direct conv as implicit GEMM on the TensorEngine):
  - x stored column-padded [ci, 56, 58] bf16 in SBUF (pad cols zeroed,
    +1-element guards at both flat ends) so every 3x3 tap is a contiguous
    1-D shifted window (the matmul ISA requires single-free-dim operands).
  - row taps at the image top/bottom use clipped row ranges; the center tap
    is issued first per ci-block pass (full coverage, start=True), the
    clipped taps accumulate -> exact zero-padding semantics.
  - A_w applied on-device as a VectorE broadcast multiply during the
    f32 -> bf16 weight cast.
  - compute dtype bf16 (fp32 PSUM accumulate), rel-err vs fp32 ~3e-3.
  - thin SE-branch matmul groups (16-wide) are interleaved with dense
    128x128 main-conv groups to keep the PE activity monitor from
    re-throttling the clock (HAM).
  - main-conv PSUM tiles drain to SBUF; the attention multiply is fused
    when `a` for that tile is already available, otherwise applied in a
    deferred VectorE pass before the output DMA.
"""

import numpy as np

import concourse.bass as bass  # noqa: F401
import concourse.mybir as mybir
import concourse.tile as tile
from concourse import bacc
from concourse.bass_utils import run_bass_kernel_spmd
from concourse.masks import make_identity

B, C, H, W = 8, 256, 56, 56
HW = H * W
WP = W + 2                      # padded row width (c=0 left pad, c=57 right pad)
HWP = H * WP                    # 3248
CMID = 16
N_CORES = 8
RT = 8                          # output rows per PSUM tile
NT = H // RT                    # 7
F32 = mybir.dt.float32
BF16 = mybir.dt.bfloat16

# center tap first within each ci-block pass
TAPS = [(0, 0)] + [
    (dh, dw) for dh in (-1, 0, 1) for dw in (-1, 0, 1) if (dh, dw) != (0, 0)
]


def _rows(r0, dh):
    """Clipped local row range [rl, rh) of a tile at base row r0 for row-tap dh."""
    return max(0, -dh - r0), min(RT, H - dh - r0)


def build():
    nc = bacc.Bacc("TRN2", target_bir_lowering=False, debug=False, num_devices=N_CORES)

    # x pre-padded on host: [ci-block, 128, 1 + 56*58 + 1] bf16, zero pad
    # columns and flat-end guards baked in
    x_d = nc.dram_tensor("xpad", [2, 128, HWP + 2], BF16, kind="ExternalInput").ap()
    # transposed on host: [ci, kh, kw, co]
    wt_d = nc.dram_tensor("weightT", [C, 3, 3, C], BF16, kind="ExternalInput").ap()
    aw_d = nc.dram_tensor("A_w", [1, C, 3, 3], F32, kind="ExternalInput").ap()
    # SE weights pre-packed on host: kw groups at 32-col strides per kh
    w1p_d = nc.dram_tensor("se_w1P", [2, 128, 3 * 96], BF16, kind="ExternalInput").ap()
    w2p_d = nc.dram_tensor("se_w2P", [CMID, 3 * 96], BF16, kind="ExternalInput").ap()
    # output in padded layout [ci-block, 128, 56*58]; host strips pad cols
    out_d = nc.dram_tensor("outp", [2, 128, HWP], F32, kind="ExternalOutput").ap()

    wt_v = wt_d.rearrange("(b p) kh kw co -> b p (kh kw co)", b=2)      # [2,128,2304]
    aw_v = aw_d[0].rearrange("(b p) kh kw -> b p (kh kw)", b=2)         # [2,128,9]

    with tile.TileContext(nc) as tc:
        with (
            tc.tile_pool(name="sb", bufs=1) as sb,
            tc.tile_pool(name="ps", space="PSUM", bufs=2) as ps,
        ):
            asb = sb.tile([128, HWP], F32, name="asb")
            osb = [sb.tile([128, HWP], F32, name=f"osb{c}") for c in range(2)]
            # +2: one guard element at each flat end (dw=+-1 at image corners)
            xs = [sb.tile([128, HWP + 2], BF16, name=f"xs{i}") for i in range(2)]
            wrt = [sb.tile([128, 2304], BF16, name=f"wrt{i}") for i in range(2)]
            aw = [sb.tile([128, 9], F32, name=f"aw{i}") for i in range(2)]
            wmod = [sb.tile([128, 9 * 256], BF16, name=f"wmod{i}") for i in range(2)]
            mid = sb.tile([CMID, HWP + 2], BF16, name="mid")
            identE = sb.tile([96, CMID], BF16, name="identE")
            identTE = sb.tile([96, 128], BF16, name="identTE")
            u1pp = [sb.tile([96, RT * WP], BF16, name=f"u1pp{k}") for k in range(2)]
            u2pp = [sb.tile([96, RT * WP], BF16, name=f"u2pp{k}") for k in range(2)]
            w2pack = sb.tile([CMID, 3 * 96], BF16, name="w2pack")
            w1pack = [sb.tile([128, 3 * 96], BF16, name=f"w1pack{i}") for i in range(2)]

            # -------- loads --------
            # x first (the PE's first dependency); one ci-block per HWDGE queue
            nc.scalar.dma_start(xs[0], x_d[0])
            nc.sync.dma_start(xs[1], x_d[1])
            nc.scalar.dma_start(w1pack[0], w1p_d[0])
            nc.sync.dma_start(w1pack[1], w1p_d[1])
            nc.scalar.dma_start(wrt[0], wt_v[0])
            nc.sync.dma_start(wrt[1], wt_v[1])
            nc.gpsimd.dma_start(w2pack, w2p_d)
            for i in range(2):
                nc.gpsimd.dma_start(aw[i], aw_v[i])

            # -------- x pad + cast (DVE, ahead of weight prep) --------
            def pad_memset(tl, np_):
                nc.vector.memset(tl[:np_, 0:2], 0.0)
                nc.vector.memset(tl[:np_, HWP : HWP + 2], 0.0)
                pads = tl[:np_, 1 + W + 1 : 1 + W + 1 + (H - 1) * WP].rearrange(
                    "p (h c) -> p h c", c=WP
                )
                nc.vector.memset(pads[:, :, 0:2], 0.0)

            # -------- prep (VectorE only, no PE) --------
            pad_memset(mid, CMID)
            for k in range(2):
                nc.vector.memset(u1pp[k], 0.0)
                nc.vector.memset(u2pp[k], 0.0)
            # identity selectors, one copy per 32-aligned strip (matmul
            # operands must share a 32-aligned partition base)
            nc.vector.memset(identE, 0.0)
            nc.vector.memset(identTE, 0.0)
            for g in range(3):
                make_identity(nc, identE[32 * g : 32 * g + CMID, :], nomemset=True)
                nc.vector.tensor_copy(
                    identTE[32 * g : 32 * g + CMID, :].rearrange(
                        "p (r c) -> p r c", c=CMID
                    ),
                    identE[32 * g : 32 * g + CMID, :]
                    .unsqueeze(1)
                    .broadcast_to([CMID, 8, CMID]),
                )
            for i in range(2):
                # wmod[ci, k, co] = weightT[ci, k, co] * A_w[ci, k]  (cast to bf16)
                nc.vector.tensor_mul(
                    wmod[i].rearrange("p (k co) -> p k co", co=256),
                    wrt[i].rearrange("p (k co) -> p k co", co=256),
                    aw[i].unsqueeze(2).broadcast_to([128, 9, 256]),
                )

            mid_v = mid[:, 1 : 1 + HWP].rearrange("p (h c) -> p h c", c=WP)
            TFv = RT * WP
            wmod_v = [wmod[i].rearrange("p (k co) -> p k co", co=256) for i in range(2)]

            # -------- conv group emitters --------
            # SE convs: the 3 kw taps are packed into the stationary columns
            # (48 = 3 kw x 16 ch), then reduced across partition groups with
            # +-1-shifted identity matmuls. Junk in pad columns only.
            def conv1_pack(t):
                r0 = t * RT
                mps = ps.tile([96, TFv], F32, name="mps96", tag="pack", bufs=3)
                n_mm = 0
                for i in range(2):
                    for dh in (0, -1, 1):
                        kh = dh + 1
                        rl, rh = _rows(r0, dh)
                        n_mm += 1
                        nc.tensor.matmul(
                            mps[:, rl * WP : rh * WP],
                            w1pack[i][:, kh * 96 : (kh + 1) * 96],
                            xs[i][:, 1 + (r0 + rl + dh) * WP :][:128, : (rh - rl) * WP],
                            start=(n_mm == 1),
                            stop=(n_mm == 6),
                        )
                u = u1pp[t % 2]
                # drain each kw strip with its +-1 column shift baked in, so
                # one K=96 selector matmul can reduce without further shifts
                ident = mybir.ActivationFunctionType.Identity
                nc.vector.tensor_copy(u[0:16, 1:TFv], mps[0:16, 0 : TFv - 1])
                nc.scalar.activation(u[32:48, :], mps[32:48, :], ident)
                nc.scalar.activation(u[64:80, 0 : TFv - 1], mps[64:80, 1:TFv], ident)
                return u

            def conv1_sel(t, u):
                r0 = t * RT
                mid_ps = ps.tile([CMID, TFv], F32, name="mid_ps", tag="red", bufs=2)
                nc.tensor.matmul(mid_ps, identE, u, start=True, stop=True)
                mpv = mid_ps.rearrange("p (h c) -> p h c", c=WP)
                nc.scalar.activation(
                    mid_v[:, r0 : r0 + RT, 1 : W + 1],
                    mpv[:, :, 1 : W + 1],
                    mybir.ActivationFunctionType.Relu,
                )

            def conv2_pack(t):
                r0 = t * RT
                ups = ps.tile([96, TFv], F32, name="u2ps", tag="pack", bufs=3)
                n_mm = 0
                for dh in (0, -1, 1):
                    kh = dh + 1
                    rl, rh = _rows(r0, dh)
                    n_mm += 1
                    nc.tensor.matmul(
                        ups[:, rl * WP : rh * WP],
                        w2pack[:, kh * 96 : (kh + 1) * 96],
                        mid[:, 1 + (r0 + rl + dh) * WP :][:CMID, : (rh - rl) * WP],
                        start=(n_mm == 1),
                        stop=(n_mm == 3),
                    )
                u = u2pp[t % 2]
                ident = mybir.ActivationFunctionType.Identity
                nc.vector.tensor_copy(u[0:16, 1:TFv], ups[0:16, 0 : TFv - 1])
                nc.scalar.activation(u[32:48, :], ups[32:48, :], ident)
                nc.scalar.activation(u[64:80, 0 : TFv - 1], ups[64:80, 1:TFv], ident)
                return u

            def conv2_sel(t, u):
                r0 = t * RT
                aps = ps.tile([128, TFv], F32, name="aps", tag="red", bufs=2)
                nc.tensor.matmul(aps, identTE, u, start=True, stop=True)
                nc.scalar.activation(
                    asb[:, r0 * WP : (r0 + RT) * WP],
                    aps,
                    mybir.ActivationFunctionType.Sigmoid,
                )

            def main_group(t, c, fused):
                r0 = t * RT
                yps = ps.tile([128, RT * WP], F32, name="yps", tag="yps", bufs=3)
                n_mm = 0
                for i in range(2):
                    for dh, dw in TAPS:
                        k = (dh + 1) * 3 + (dw + 1)
                        rl, rh = _rows(r0, dh)
                        n_mm += 1
                        nc.tensor.matmul(
                            yps[:, rl * WP : rh * WP],
                            wmod_v[i][:, k, c * 128 : (c + 1) * 128],
                            xs[i][:, 1 + (r0 + rl + dh) * WP + dw :][:128, : (rh - rl) * WP],
                            start=(n_mm == 1),
                            stop=(n_mm == 18),
                        )
                dst = osb[c][:, r0 * WP : (r0 + RT) * WP]
                if fused:
                    nc.vector.tensor_mul(dst, yps, asb[:, r0 * WP : (r0 + RT) * WP])
                    q = nc.sync if (t + c) % 2 == 0 else nc.scalar
                    q.dma_start(out_d[c][:, r0 * WP : (r0 + RT) * WP], dst)
                else:
                    nc.vector.tensor_copy(dst, yps)

            # -------- interleaved schedule --------
            # main groups in issue order; SE groups threaded between them so
            # the PE never sees a long run of thin (16-wide) matmuls.
            main_q = [(t, c) for t in range(NT) for c in range(2)]
            mq = iter(main_q)
            deferred = []
            sig_done = [False] * NT

            def emit_main(n, fused_allowed):
                for _ in range(n):
                    tc_ = next(mq, None)
                    if tc_ is None:
                        return
                    t, c = tc_
                    if sig_done[t] and fused_allowed:
                        main_group(t, c, fused=True)
                    else:
                        main_group(t, c, fused=False)
                        deferred.append((t, c))

            def flush_deferred():
                rest = []
                for t, c in deferred:
                    if not sig_done[t]:
                        rest.append((t, c))
                        continue
                    r0 = t * RT
                    dst = osb[c][:, r0 * WP : (r0 + RT) * WP]
                    nc.vector.tensor_mul(dst, dst, asb[:, r0 * WP : (r0 + RT) * WP])
                    q = nc.sync if (t + c) % 2 == 0 else nc.scalar
                    q.dma_start(out_d[c][:, r0 * WP : (r0 + RT) * WP], dst)
                deferred[:] = rest

            u_prev = None
            for t in range(NT):
                u = conv1_pack(t)
                if u_prev is not None:
                    conv1_sel(t - 1, u_prev)
                    emit_main(1, fused_allowed=False)
                u_prev = u
            conv1_sel(NT - 1, u_prev)
            emit_main(1, fused_allowed=False)
            u_prev = None
            for t in range(NT):
                u = conv2_pack(t)
                if u_prev is not None:
                    conv2_sel(t - 1, u_prev)
                    sig_done[t - 1] = True
                    if t % 2 == 1:
                        emit_main(1, fused_allowed=True)
                        flush_deferred()
                u_prev = u
            conv2_sel(NT - 1, u_prev)
            sig_done[NT - 1] = True
            # remaining main groups: `a` is fully available, fuse the multiply
            emit_main(len(main_q), fused_allowed=True)
            flush_deferred()

    nc.compile()
    return nc


_NC = None


def make_in_maps(x, weight, A_w, se_w1, se_w2):
    import ml_dtypes

    bf16 = ml_dtypes.bfloat16
    x = np.asarray(x, dtype=np.float32)
    # pre-padded x: [B, ci-block, 128, guard + 56*58 + guard] with zero pad
    # columns (c=0, c=57) and guards
    xpad = np.zeros((B, 2, 128, HWP + 2), dtype=bf16)
    xv = xpad[:, :, :, 1 : 1 + HWP].reshape(B, 2, 128, H, WP)
    xv[:, :, :, :, 1 : W + 1] = x.reshape(B, 2, 128, H, W).astype(bf16)

    weightT = np.ascontiguousarray(
        np.asarray(weight, dtype=np.float32).transpose(1, 2, 3, 0).astype(bf16)
    )
    A_w = np.ascontiguousarray(np.asarray(A_w, dtype=np.float32))

    # SE weights pre-packed: kw groups at 32-col strides per kh slice
    w1T = np.asarray(se_w1, dtype=np.float32).transpose(1, 2, 3, 0)  # [ci,kh,kw,16]
    w1P = np.zeros((2, 128, 3, 3, 32), dtype=bf16)
    w1P[:, :, :, :, :CMID] = w1T.reshape(2, 128, 3, 3, CMID).astype(bf16)
    w1P = np.ascontiguousarray(w1P.reshape(2, 128, 3 * 96))
    w2P = np.zeros((CMID, 3, 3, 32), dtype=bf16)
    w2P[:, :, :, :CMID] = (
        np.asarray(se_w2, dtype=np.float32)[0].astype(bf16)[:, :, :, None]
    )
    w2P = np.ascontiguousarray(w2P.reshape(CMID, 3 * 96))

    in_maps = [
        {
            "xpad": np.ascontiguousarray(xpad[b]),
            "weightT": weightT,
            "A_w": A_w,
            "se_w1P": w1P,
            "se_w2P": w2P,
        }
        for b in range(B)
    ]
    return in_maps


def kernel(x, weight, A_w, se_w1, se_w2):
    global _NC
    if _NC is None:
        _NC = build()
    in_maps = make_in_maps(x, weight, A_w, se_w1, se_w2)
    res = run_bass_kernel_spmd(_NC, in_maps, list(range(N_CORES)))
    out = np.stack([res.results[b]["outp"] for b in range(B)], axis=0)
    # strip pad columns: [B,2,128,56*58] -> [B,256,56,56]
    out = out.reshape(B, 2, 128, H, WP)[:, :, :, :, 1 : W + 1].reshape(B, C, H, W)
    return np.ascontiguousarray(out)
